# revision 60
# baseline (speedup 1.0000x reference)
"""GWPooling2D forward on 8 Trainium2 NeuronCores.

y[b, c, o] = sum_k m[c, o, k] * x[b, k]   (k = 400 input pixels, o = 256)

The pooling map m depends only on the small `signal` parameter and is
computed on host exactly as in the reference. It decomposes as

    m[c] = P0 + E[c]

where P0 (256 x 400) is the signal-independent resampling map (expm(0)=I
pushed through the same crop/roll/FFT pipeline) shared by all 16 channels,
and E[c] is the small per-channel correction (||E|| ~ 0.17 ||m||).

Device work per core (1024-batch shard, data parallel across 8 cores), all
matmuls fp8e4m3 DoubleRow (0.5 cycles/row, K=400 packed as [100, chunk=2,
dr=2]):
  yE*s_e = x_hi @ (E*s_e)^T                         4096 cols, stored fp8
  yP*s_p = x_hi @ (P1+P2)^T + x_lo' @ P3^T           256 cols, stored bf16
where x = x_hi + x_lo/64 is a two-level fp8 split of x, and P1 = fp8(P0*s_p),
P2 = fp8(P0*s_p - P1), P3 = fp8((P1+P2)/64) give an effectively-bf16 P0.
yE is ~17% of y, so its fp8 store noise is ~0.6% of y; the host computes
y = yP/s_p + yE/s_e (rel err ~0.9% vs the 2e-2 gate).

Schedule (TimelineSim cost model, 28.1us vs 30.6us baseline): the hard
floor is PSUM evacuation - every output element crosses DVE or ACT at
1 elem/cycle/partition from f32 PSUM (~19.3us busy each) - with DMA busy
(21us) just under it.  The kernel is organized so both copy engines run
back-to-back from ~4.4us:
 - quarter-major phases (1024 output columns x 8 batch tiles each) so the
   first phase only needs the first E-map quarter; the very first tile is
   drained in 512-col halves (ACT then DVE) gated on two small leading
   loads (xhf covering 3 batch tiles, q0a);
 - 2-bank PSUM tiles, pool bufs=4 (all 8 banks), warmup matmuls write the
   tile the first real matmul group overwrites;
 - one copy per tile, DVE/ACT chosen greedily by modeled busy time;
 - DMA issue is spread across SP/HWDGE (loads + q2/q3 stores) and the
   otherwise idle Pool/SWDGE (q0a load, q0/q1 stores, outP stores) because
   each HWDGE issue holds the shared HWDGE device 625ns;
 - yP pairs are slotted into phase q2 between E tiles; the final two batch
   tiles drain as four 512-col copies, column-half-major, so both engines
   finish together and the tail stores are short.
"""

import numpy as np
import scipy.linalg

import concourse.bass as bass
import concourse.bacc as bacc
import concourse.mybir as mybir
import concourse.tile as tile
from concourse.bass_utils import run_bass_kernel_spmd
import ml_dtypes

C = 16
P = (24, 24)
NI = (20, 20)
NO = (16, 16)
B = 8192
NCORES = 8
BS = B // NCORES              # 1024 batch rows per core
K = NI[0] * NI[1]             # 400 contraction
O = NO[0] * NO[1]             # 256 output positions per channel
CO = C * O                    # 4096 (c,o) output columns
BT = 128                      # batch tile (PSUM partitions)
OT = 512                      # matmul free-dim tile (one PSUM bank fp32)
K0 = 256                      # DoubleRow chunk 0 (2 x 128 partitions)
K1 = K - K0                   # 144 = 2 x 72 partitions
EQ = 1024                     # E-map load quarter (columns)
TW = 2048                     # PSUM tile width (4 banks) = copy/store width

F8 = ml_dtypes.float8_e4m3
BF16 = ml_dtypes.bfloat16


# ---------------------------------------------------------------- host map ---

def _hann(n):
    return 0.5 * (1.0 - np.cos(2.0 * np.pi * np.arange(n) / n))


def _signal_to_spectrum(signal):
    n0, n1 = signal.shape[-2], signal.shape[-1]
    window = _hann(n0)[:, None] * _hann(n1)[None, :]
    rx = np.arange((-n0) // 2 + 1, n0 // 2 + 1)[:, None]
    ry = np.arange((-n1) // 2 + 1, n1 // 2 + 1)[None, :]
    r = (1 + rx * rx + ry * ry).astype(np.float64)
    wf = np.roll(np.fft.fft2(signal), (n0 // 2, n1 // 2), (-2, -1)) / r / 5.0
    wt = np.fft.ifft2(np.roll(wf, (-(n0 // 2), -(n1 // 2)), (-2, -1))) * window
    return np.roll(np.fft.fft2(wt), (n0 // 2, n1 // 2), (-2, -1))


def _gw2d_algebra(w):
    p0, p1 = w.shape[-2], w.shape[-1]
    pad = [(0, 0)] * (w.ndim - 2) + [(p1 // 2, p1 // 2), (p0 // 2, p0 // 2)]
    wp = np.pad(w, pad)
    ia = np.arange(p0)[:, None] + np.arange(p0)[None, :]
    jb = np.arange(p1)[:, None] + np.arange(p1)[None, :]
    ws = wp[..., ia[:, None, :, None], jb[None, :, None, :]]
    ws = ws[..., ::-1, ::-1, :, :]
    kx = np.arange((-p0) // 2 + 1, p0 // 2 + 1)[:, None]
    ky = np.arange((-p1) // 2 + 1, p1 // 2 + 1)[None, :]
    return -1j * (ws[..., 0, :, :, :, :] * kx + ws[..., 1, :, :, :, :] * ky)


def _transform_to_map(t):
    p0, p1 = t.shape[-2], t.shape[-1]
    di = (p0 - NI[0], p1 - NI[1])
    do = (p0 - NO[0], p1 - NO[1])
    x = t[..., do[0] // 2 + 1:(-do[0]) // 2 + 1, do[1] // 2 + 1:(-do[1]) // 2 + 1,
          di[0] // 2 + 1:(-di[0]) // 2 + 1, di[1] // 2 + 1:(-di[1]) // 2 + 1]
    x = np.roll(x, (NO[0] // 2 + 1, NO[1] // 2 + 1, NI[0] // 2 + 1, NI[1] // 2 + 1),
                (-4, -3, -2, -1))
    return np.fft.fft2(np.fft.ifft2(x, axes=(-2, -1)), axes=(-4, -3)).real


def compute_mf(signal):
    """signal (C,2,24,24) -> pooling matrix (CO=4096, K=400) float32."""
    spectrum = _signal_to_spectrum(signal.astype(np.float64))
    p0, p1 = spectrum.shape[-2], spectrum.shape[-1]
    a = _gw2d_algebra(spectrum)
    n = p0 * p1
    mat = a.reshape(a.shape[:-4] + (n, n))
    t = np.stack([scipy.linalg.expm(mat[i]) for i in range(mat.shape[0])])
    t = t.reshape(t.shape[:-2] + (p0, p1, p0, p1))
    m = _transform_to_map(t)
    return m.reshape(CO, K).astype(np.float32)


_P0 = None


def compute_p0():
    """Signal-independent part of the map: expm(0)=I through the same
    crop/roll/FFT pipeline. (256, 400) float64."""
    global _P0
    if _P0 is None:
        t_id = np.eye(P[0] * P[1], dtype=np.complex128).reshape(
            1, P[0], P[1], P[0], P[1])
        _P0 = _transform_to_map(t_id).reshape(O, K)
    return _P0


# ------------------------------------------------------------ device kernel ---

_built = None

# schedule knobs (tuned against the TimelineSim cost model)
WARMUPS = 22
HEAD_ENG = ("act", "dve")     # engines for the first tile's two half copies
B1_ENG = "dve"                # forced engine for tile (q0, b1)
TAIL_ENG = (None, None, None, None)   # forced engines for the 4 tail halves
B1_HALVES = False             # drain tile (q0, b1) as two 512-col halves
XHF_BTS = 3
TAIL_FULL = False                   # batch tiles covered by the leading xhf slice
P_Q = (2, 2, 2, 2)            # which quarter-phase hosts P pair i
P_B = (0, 3, 5, 7)            # after which batch tile within that phase
Q0B_POOL = False              # issue q0's second half via Pool/SWDGE too
BT_ROT = (0, 0, 0, 0)         # per-phase batch-tile rotation (even values)
STPOOL = 16
POOL_STORE_Q = (0, 1)         # quarter-phases whose stores issue via Pool
SWAP_COPIES = ()              # copy indices whose greedy engine is flipped
TAIL_ST1 = "pool"             # issuer of the first tail col-half store
TAIL_ST1_SPLIT = False        # split tail store1 per batch tile (Pool+SP)


def _build():
    global _built
    if _built is not None:
        return _built
    nc = bacc.Bacc(dynamic_dma_scratch_size=16384)
    f32 = mybir.dt.float32
    bf16 = mybir.dt.bfloat16
    f8 = mybir.dt.float8e4
    DR = mybir.MatmulPerfMode.DoubleRow

    KP = 100                      # partitions: K=400 as 2 chunks x (100,2) DR
    # x: fp8 DoubleRow packing [100, chunk, dr, B]; hi and lo tensors
    xh_d = nc.declare_dram_parameter("xh", (KP, 2, 2, BS), f8, isOutput=False)
    xhf_d = nc.declare_dram_parameter("xhf", (KP, 2, 2, XHF_BTS * BT), f8,
                                      isOutput=False)
    xl_d = nc.declare_dram_parameter("xl", (KP, 2, 2, BS), f8, isOutput=False)
    # P0 hi/lo/lo-shifted stack: cols [P1 | P2 | P3]
    pc_d = nc.declare_dram_parameter("pc", (KP, 2, 2, 3 * O), f8, isOutput=False)
    e8_d = nc.declare_dram_parameter("e8", (KP, 2, 2, CO), f8, isOutput=False)
    outE_d = nc.declare_dram_parameter("outE", (BS, CO), f8, isOutput=True)
    outP_d = nc.declare_dram_parameter("outP", (BS, O), bf16, isOutput=True)

    NB = BS // BT                 # 8 batch tiles

    with tile.TileContext(nc) as tc:
        with tc.tile_pool(name="inpool", bufs=1) as inpool, \
             tc.tile_pool(name="stpool", bufs=STPOOL) as stpool, \
             tc.tile_pool(name="ypool", bufs=1) as ypool, \
             tc.tile_pool(name="pepool", bufs=4, space="PSUM") as pepool:

            # PE warmup: burn most of the p-state ramp during the load
            # phase.  The warmup matmuls write into the PSUM tile the first
            # real E tile will overwrite (start=True clears), so no pool slot
            # is held beyond it.
            warm = inpool.tile([128, 2, 256], f8, name="warm")
            nc.vector.memset(warm[:], 0.0)
            warm_ps = pepool.tile([BT, EQ], f32, name="ps")
            for _ in range(WARMUPS):
                nc.tensor.matmul(warm_ps[:, :256], warm[:, :, :BT], warm[:],
                                 start=True, stop=True, perf_mode=DR)

            # ---- loads (SP/HWDGE), ordered so quarter 0 starts early: a
            # small x slice for batch tiles 0-1 and the first 512 E columns
            # land first, then everything else in use order.
            xhf = inpool.tile([KP, 2, 2, XHF_BTS * BT], f8, name="xhf")
            nc.sync.dma_start(xhf[:], xhf_d[:])

            def xhi(b):
                """hi-x tile + column slice for batch tile b."""
                if b < XHF_BTS:
                    return xhf, slice(b * BT, (b + 1) * BT)
                return xh, slice(b * BT, (b + 1) * BT)

            e8q = [None] * 4

            def load_eq(q, split=False):
                t = inpool.tile([KP, 2, 2, EQ], f8, tag=f"e8q{q}",
                                name=f"e8q{q}")
                if split:
                    # first half issued via Pool/SWDGE so it lands in
                    # parallel with SP's leading loads
                    s = EQ // 2
                    nc.gpsimd.dma_start(t[:, :, :, :s],
                                        e8_d[:, :, :, q * EQ:q * EQ + s])
                    eng2 = nc.gpsimd if Q0B_POOL else nc.sync
                    eng2.dma_start(t[:, :, :, s:],
                                   e8_d[:, :, :, q * EQ + s:(q + 1) * EQ])
                else:
                    nc.sync.dma_start(t[:], e8_d[:, :, :, q * EQ:(q + 1) * EQ])
                e8q[q] = t

            load_eq(0, split=True)
            xh = inpool.tile([KP, 2, 2, BS], f8, name="xh")
            nc.sync.dma_start(xh[:], xh_d[:])
            load_eq(1)
            xl = inpool.tile([KP, 2, 2, BS], f8, name="xl")
            nc.sync.dma_start(xl[:], xl_d[:])
            pc = inpool.tile([KP, 2, 2, 3 * O], f8, name="pc")
            nc.sync.dma_start(pc[:], pc_d[:])
            load_eq(2)
            load_eq(3)

            yps = ypool.tile([BT, NB, O], bf16, name="yps")

            # static greedy DVE/ACT balance (cost-model rates incl per-copy
            # access overhead)
            ebusy = {"dve": 0.0, "act": 0.0}

            copy_idx = [0]

            def do_copy(dst, src, nels, eng=None):
                i = copy_idx[0]
                copy_idx[0] += 1
                if eng is None:
                    eng = "dve" if ebusy["dve"] <= ebusy["act"] else "act"
                    if i in SWAP_COPIES:
                        eng = "act" if eng == "dve" else "dve"
                if eng == "dve":
                    ebusy["dve"] += nels * 1.0417 + 125.0
                    nc.vector.tensor_copy(dst, src)
                else:
                    ebusy["act"] += nels * 0.8333 + 185.0
                    nc.scalar.copy(dst, src)
                return eng

            def e_mm(ps, psl, xa, xs, q, off, width):
                for c in range(2):
                    nc.tensor.matmul(
                        ps[:, psl:psl + width], xa[:, c, :, xs],
                        e8q[q][:, c, :, off:off + width],
                        start=(c == 0), stop=(c == 1), perf_mode=DR,
                    )

            def e_tile(q, b, st2, jj, halves=False, eng=None):
                """1024 cols of quarter q for batch tile b: 4 DoubleRow
                matmuls into a 2-bank PSUM tile, drained into st2[:, jj, :].
                halves=True uses one PSUM tile + copy per 512-col half (for
                pipeline head/tail ramps)."""
                xa, xs = xhi(b)
                if halves:
                    for j in range(2):
                        if q == 0 and b == 0 and j == 0:
                            ps = warm_ps
                        else:
                            ps = pepool.tile([BT, EQ], f32, name="ps")
                        off = j * OT
                        e_mm(ps, 0, xa, xs, q, off, OT)
                        do_copy(st2[:, jj, off:off + OT], ps[:, :OT], OT,
                                eng=HEAD_ENG[j] if q == 0 and b == 0 else eng)
                else:
                    ps = pepool.tile([BT, EQ], f32, name="ps")
                    for j in range(2):
                        e_mm(ps, j * OT, xa, xs, q, j * OT, OT)
                    do_copy(st2[:, jj, :], ps[:], EQ, eng=eng)

            def p_pair(pair):
                """yP for batch tiles 2*pair, 2*pair+1 accumulated in one PSUM
                bank; 12 DoubleRow matmuls -> one 512-col copy into yps."""
                ps = pepool.tile([BT, EQ], f32, name="ps")
                for b2 in range(2):
                    b = pair * 2 + b2
                    sl = b2 * O
                    for r in range(3):
                        if r < 2:
                            xa, xps = xhi(b)
                        else:
                            xa, xps = xl, slice(b * BT, (b + 1) * BT)
                        for c in range(2):
                            nc.tensor.matmul(
                                ps[:, sl:sl + O], xa[:, c, :, xps],
                                pc[:, c, :, r * O:(r + 1) * O],
                                start=(r == 0 and c == 0),
                                stop=(r == 2 and c == 1), perf_mode=DR,
                            )
                do_copy(yps[:, 2 * pair:2 * pair + 2, :], ps[:, :2 * O], 2 * O)

            # quarter-major phases: all 8 batch tiles of one 1024-col quarter,
            # stores of 2 batch tiles each.  q0/q1 stores ride Pool/SWDGE,
            # q2/q3 SP/HWDGE; P pairs interleave into q2 once xl/pc landed.
            for q in range(4):
                store_eng = nc.gpsimd if q in POOL_STORE_Q else nc.sync
                for bb0 in range(NB // 2):
                    bb = (bb0 + BT_ROT[q] // 2) % (NB // 2)
                    st2 = stpool.tile([BT, 2, EQ], f8, name="st2")
                    last = (q == 3 and bb0 == NB // 2 - 1)
                    if last:
                        if TAIL_FULL:
                            for b2 in range(2):
                                e_tile(q, 2 * bb + b2, st2, b2)
                        else:
                            # drain the final two batch tiles in 512-col
                            # halves, column-half-major and engines
                            # alternating, so both engines finish together
                            # and the tail stores are short
                            for j in range(2):
                                for b2 in range(2):
                                    b = 2 * bb + b2
                                    ps = pepool.tile([BT, EQ], f32, name="ps")
                                    xa, xs = xhi(b)
                                    e_mm(ps, 0, xa, xs, q, j * OT, OT)
                                    do_copy(st2[:, b2, j * OT:(j + 1) * OT],
                                            ps[:, :OT], OT,
                                            eng=TAIL_ENG[2 * j + b2])
                    else:
                        for b2 in range(2):
                            b = 2 * bb + b2
                            e_tile(q, b, st2, b2,
                                   halves=(q == 0 and (b == 0 or
                                           (b == 1 and B1_HALVES))),
                                   eng=(B1_ENG if q == 0 and b == 1 else None))
                            for pi in range(4):
                                if P_Q[pi] == q and P_B[pi] == b:
                                    p_pair(pi)
                    dst = outE_d[bb * 2 * BT:(bb + 1) * 2 * BT,
                                 q * EQ:(q + 1) * EQ]
                    dstr = dst.rearrange("(j p) o -> p j o", p=BT)
                    if last:
                        # final stores by column half: first half (both batch
                        # tiles) as soon as its two copies land, the second
                        # right after the last copies
                        st1_eng = {"pool": nc.gpsimd, "sp": store_eng,
                                   "act": nc.scalar}[TAIL_ST1]
                        if TAIL_ST1_SPLIT:
                            # per-bt split: the Pool-issued half's transfer is
                            # short (182ns) so the final SP store's transfer
                            # does not FIFO-queue behind it on the DMA engines
                            st1_eng.dma_start(dstr[:, 1:2, :OT],
                                              st2[:, 1:2, :OT])
                            store_eng.dma_start(dstr[:, 0:1, :OT],
                                                st2[:, 0:1, :OT])
                        else:
                            st1_eng.dma_start(dstr[:, :, :OT], st2[:, :, :OT])
                        store_eng.dma_start(dstr[:, :, OT:], st2[:, :, OT:])
                    else:
                        store_eng.dma_start(dstr, st2[:])
                if q == 2:
                    nc.gpsimd.dma_start(
                        outP_d[:4 * BT].rearrange("(j p) o -> p j o", p=BT),
                        yps[:, :4, :])
                    nc.gpsimd.dma_start(
                        outP_d[4 * BT:].rearrange("(j p) o -> p j o", p=BT),
                        yps[:, 4:, :])
    nc.compile()
    _built = nc
    return nc


SX = 64.0


def _prep_host(x, signal):
    XHF = XHF_BTS
    """Host-side factorization + quantization. Returns per-core input maps
    and the dequantization scales (s_e, s_p)."""
    mf = compute_mf(np.asarray(signal))                     # (4096, 400)
    p0 = compute_p0()                                       # (256, 400) f64
    e = mf.astype(np.float64).reshape(C, O, K) - p0[None]
    ef = e.reshape(CO, K)

    # E path scale: keeps E*s inside fp8 range and (with 8-sigma slack for
    # x ~ N(0,1)) the yE accumulator inside +-240 at the fp8 store
    row_norm = np.sqrt((ef * ef).sum(axis=1)).max()
    s_e = min(200.0 / np.abs(ef).max(), 200.0 / (8.0 * row_norm))
    e8 = (ef * s_e).astype(np.float32).astype(F8)           # (4096, 400)
    e8c0 = np.ascontiguousarray(
        e8[:, :K0].reshape(CO, 2, K0 // 2).transpose(2, 1, 0))   # (128,2,4096)
    e8c1 = np.ascontiguousarray(
        e8[:, K0:].reshape(CO, 2, K1 // 2).transpose(2, 1, 0))   # (72,2,4096)

    # P path: hi/lo fp8 split of P0 (and of x), shared x_hi with the E path
    s_p = 200.0 / np.abs(p0).max()
    p1 = (p0 * s_p).astype(np.float32).astype(F8)
    p2 = (p0 * s_p - p1.astype(np.float64)).astype(np.float32).astype(F8)
    p3 = ((p1.astype(np.float32) + p2.astype(np.float32)) / SX).astype(F8)
    p123 = np.concatenate(
        [p1.astype(np.float32), p2.astype(np.float32), p3.astype(np.float32)],
        axis=0)                                             # (3*O, K)
    pc0 = np.ascontiguousarray(
        p123.astype(F8)[:, :K0].reshape(3 * O, 2, K0 // 2).transpose(2, 1, 0))
    pc1 = np.ascontiguousarray(
        p123.astype(F8)[:, K0:].reshape(3 * O, 2, K1 // 2).transpose(2, 1, 0))

    xT = np.asarray(x).reshape(B, K).T.astype(np.float32)   # (400, 8192)
    x_hi = xT.astype(F8)
    x_lo = ((xT - x_hi.astype(np.float32)) * SX).astype(F8)

    def pack4(arr):
        # (400, N) -> (100, chunk=2, dr=2, N): row 200c + 100d + i
        return np.ascontiguousarray(
            arr.reshape(2, 2, 100, arr.shape[1]).transpose(2, 0, 1, 3))

    xh = pack4(x_hi)
    xl = pack4(x_lo)
    e8p = pack4(np.ascontiguousarray(e8.T))                 # (100,2,2,4096)
    pcp = pack4(np.ascontiguousarray(p123.astype(F8).T))    # (100,2,2,768)

    in_maps = []
    for i in range(NCORES):
        bs = slice(i * BS, (i + 1) * BS)
        in_maps.append({
            "xh": np.ascontiguousarray(xh[:, :, :, bs]),
            "xhf": np.ascontiguousarray(
                xh[:, :, :, bs][:, :, :, :XHF * BT]),
            "xl": np.ascontiguousarray(xl[:, :, :, bs]),
            "pc": pcp,
            "e8": e8p,
        })
    return in_maps, s_e, s_p


def _run(x, signal, **spmd_kwargs):
    nc = _build()
    in_maps, s_e, s_p = _prep_host(x, signal)
    res = run_bass_kernel_spmd(nc, in_maps, list(range(NCORES)), **spmd_kwargs)
    parts = []
    for r in res.results:
        yE = r["outE"].astype(np.float32).reshape(BS, C, O) / s_e
        yP = r["outP"].astype(np.float32) / s_p
        parts.append(yE + yP[:, None, :])
    y = np.concatenate(parts, axis=0)
    return y.reshape(B, C, NO[0], NO[1]), res


def kernel(x, signal):
    y, _ = _run(x, signal)
    return y
